# revision 24
# baseline (speedup 1.0000x reference)
"""BiasedMHA + GLU fused Trainium2 kernel.

Problem: out = GLU(x) + OutProj(MHA(x, attn_bias))  with
  B=8, N=1024, D=768, H=12, HD=64, fp32 inputs/outputs.

Strategy: data-parallel over batch across the 8 NeuronCores (one batch
element per core, no collectives). Per core everything is computed in a
"transposed" [channel, token] layout so every GEMM contracts the
partition dimension without any on-device activation transposes:

  xT [D, N] (host-pretransposed)    qT/kT = W.T-stationary GEMMs  [D, N]
  v via xT-stationary GEMM          -> natural [token, head*65] layout
  scoresT[k, q] = kT_h.T @ qT_h accumulated on top of PE-transposed
  attn_bias blocks (bf16 identity-matmuls straight into the scores PSUM)
  softmax over k (= partitions): no max-subtraction (|scores| <= ~8),
  denominator via an appended ones-column in v, applied after PV.
  Matmuls run in float32r (TF32-like, 11-bit mantissa, 4x faster than
  fp32 on the PE); attention processes HEAD PAIRS with the two K=64 qk
  matmuls interleaved on PE row groups 0/64 so they co-execute in the
  128x128 array and keep the HAM activity monitor fed (single-row-group
  streams let the PE clock-gate back to 1.2 GHz). Exp-dependent work is
  emitted one iteration late (software pipelining) so the in-order PE
  queue never head-of-line blocks on the ScalarE exp.

  Measured: HW exec ~286-288us per core (8 cores in parallel),
  relative error vs fp32 reference 6.2e-4.
"""

import os
import sys

for _p in ("/opt/trn_rl_repo", "/root/.axon_site/_ro/trn_rl_repo"):
    if os.path.isdir(_p) and _p not in sys.path:
        sys.path.insert(0, _p)

import numpy as np
import ml_dtypes

import concourse.bacc as bacc
import concourse.mybir as mybir
from concourse import tile
from concourse.bass_utils import run_bass_kernel_spmd
from concourse.masks import make_identity

B, N, D, H, HD = 8, 1024, 768, 12, 64
P = 128
ND = D // P           # 6 channel tiles
NN = N // P           # 8 token tiles
VW = H * (HD + 1)     # 780: v layout [token, h*(64+1)] with ones column

F32 = mybir.dt.float32
F32R = mybir.dt.float32r
BF16 = mybir.dt.bfloat16
AF = mybir.ActivationFunctionType
OP = mybir.AluOpType


def _bf16(x):
    return np.ascontiguousarray(x, dtype=np.float32).astype(ml_dtypes.bfloat16)


def _emit(nc, tc, xT, biasb, w, bvec, outT, dbg=None):
    KH = N * H // 2  # 6144 bf16 = half of k, all heads
    with tc.tile_pool(name="const", bufs=1) as constp, \
         tc.tile_pool(name="ctxT", bufs=1) as ctxp_sb, \
         tc.tile_pool(name="xp", bufs=1) as xp:

        ident = constp.tile([P, P], BF16, tag="ident", name="ident")
        make_identity(nc, ident[:])
        # PE warm-up: the first ~13us are DMA-latency-bound with zero PE
        # work, so the HAM clock-gate would hold the array at 1.2 GHz well
        # into phase B.  Burn idle cycles on an SBUF scratch tile so the
        # activity monitor un-throttles before the first real GEMM.
        wub = constp.tile([P, 512], BF16, tag="wub", name="wub")
        nc.vector.memset(wub[:], 0.0)
        with tc.tile_pool(name="psW", bufs=1, space="PSUM") as psW:
            wps = psW.tile([P, 512], F32, tag="psW", name="psW")
            for _ in range(40):
                nc.tensor.matmul(wps[:], ident[:], wub[:],
                                 start=True, stop=True, skip_group_check=True)
        bvt = {}
        for nm in ("bq", "bk", "bo"):
            t = constp.tile([P, ND], F32, tag=f"t{nm}", name=f"t{nm}")
            nc.sync.dma_start(t[:], bvec[nm].ap().rearrange("(j p) -> p j", p=P))
            bvt[nm] = t
        ones12 = constp.tile([P, H], F32, tag="ones12", name="ones12")
        nc.vector.memset(ones12[:], 1.0)

        ctxT = [ctxp_sb.tile([P, N], BF16, tag=f"cT{i}", name=f"cT{i}")
                for i in range(ND)]
        xsb = [xp.tile([P, N], BF16, tag=f"x{i}", name=f"x{i}")
               for i in range(ND)]
        # out-proj / gate weights: DMA'd here so they stream in during
        # attention; bf16 halves the traffic and SBUF footprint.
        wo = [ctxp_sb.tile([P, D], BF16, tag=f"wo{i}", name=f"wo{i}")
              for i in range(ND)]
        wg = [ctxp_sb.tile([P, D], BF16, tag=f"wg{i}", name=f"wg{i}")
              for i in range(ND)]
        bgt = constp.tile([P, ND], F32, tag="bg", name="bg")

        with tc.tile_pool(name="qkvT", bufs=1) as qkvp, \
             tc.tile_pool(name="ctxsb", bufs=1) as csbp, \
             tc.tile_pool(name="stA", bufs=3) as stpA:
            qT = [qkvp.tile([P, N], BF16, tag=f"qT{i}", name=f"qT{i}")
                  for i in range(ND)]
            kT = [qkvp.tile([P, N], BF16, tag=f"kT{i}", name=f"kT{i}")
                  for i in range(ND)]
            vsb = [qkvp.tile([P, VW], BF16, tag=f"v{t}", name=f"v{t}")
                   for t in range(NN)]
            vv = [t.rearrange("p (h c) -> p h c", c=HD + 1) for t in vsb]
            ctx_sb = [csbp.tile([HD + 1, 256], BF16, tag=f"cs{h}", name=f"cs{h}")
                      for h in range(12)]

            # ---------------- Phase B: q/k/v projections ----------------
            with tc.tile_pool(name="xTw", bufs=1) as xwp, \
                 tc.tile_pool(name="psB", bufs=2, space="PSUM") as psB:
                wsb = {nm: [xwp.tile([P, D], BF16, tag="wpool",
                                     name=f"{nm}{i}", bufs=12)
                            for i in range(ND)]
                       for nm in ("wq", "wk", "wv")}
                # critical-path DMAs first: the first q-GEMM group needs
                # wq[i]+x[i] pairs in order; bias staging has ~90us of slack.
                for i in range(ND):
                    nc.sync.dma_start(wsb["wq"][i][:], w["wq"][i * P:(i + 1) * P, :])
                    nc.sync.dma_start(xsb[i][:], xT[i * P:(i + 1) * P, :])
                for i in range(ND):
                    nc.sync.dma_start(wsb["wk"][i][:], w["wk"][i * P:(i + 1) * P, :])
                for i in range(ND):
                    nc.sync.dma_start(wsb["wv"][i][:], w["wv"][i * P:(i + 1) * P, :])
                stg0 = []
                for j in range(2):
                    st = stpA.tile([P, KH], BF16, tag="stA", name="stA")
                    nc.sync.dma_start(st[:], biasb[j * P:(j + 1) * P, 0:KH])
                    stg0.append(st.rearrange("p (k h) -> p k h", h=H))

                for nm, dst in (("wq", qT), ("wk", kT)):
                    for j in range(ND):
                        for c in range(2):
                            ps = psB.tile([P, 512], F32, tag="psB", name="psB")
                            for i in range(ND):
                                nc.tensor.matmul(
                                    ps[:], wsb[nm][i][:, j * P:(j + 1) * P],
                                    xsb[i][:, c * 512:(c + 1) * 512],
                                    start=(i == 0), stop=(i == ND - 1))
                            bt = bvt["bq" if nm == "wq" else "bk"]
                            nc.scalar.activation(dst[j][:, c * 512:(c + 1) * 512],
                                                 ps[:], AF.Identity,
                                                 bias=bt[:, j:j + 1])

                for t in range(NN):
                    nc.vector.tensor_copy(vv[t][:, :, HD], ones12[:])
                    for c, (lo, sz) in enumerate(((0, 512), (512, 256))):
                        ps = psB.tile([P, 512], F32, tag="psB", name="psB")
                        for i in range(ND):
                            nc.tensor.matmul(
                                ps[:, 0:sz], xsb[i][:, t * P:(t + 1) * P],
                                wsb["wv"][i][:, lo:lo + sz],
                                start=(i == 0), stop=(i == ND - 1))
                        h0 = lo // HD
                        nc.vector.tensor_copy(
                            vv[t][:, h0:h0 + sz // HD, 0:HD],
                            ps[:, 0:sz].rearrange("p (h c) -> p h c", c=HD))

            # ---------------- Phase C: attention ----------------
            # epochs = (qq quarter of q, K half of k); PV partials for K=0
            # park in ctx_sb, K=1 adds them back, normalizes, writes ctxT.
            with tc.tile_pool(name="stB", bufs=3) as stpB, \
                 tc.tile_pool(name="expT", bufs=3) as expp, \
                 tc.tile_pool(name="norm", bufs=2) as normp, \
                 tc.tile_pool(name="psS", bufs=3, space="PSUM") as psS, \
                 tc.tile_pool(name="psC", bufs=2, space="PSUM") as psC:
                # D/E weight prefetch, spread across attention epochs (the
                # sync DMA queue is FIFO: a burst here would delay the
                # just-in-time bias staging).
                prefetch = [lambda i=i: nc.sync.dma_start(
                                wo[i][:], w["wo"][i * P:(i + 1) * P, :])
                            for i in range(ND)]
                prefetch += [lambda i=i: nc.sync.dma_start(
                                 wg[i][:], w["wg"][i * P:(i + 1) * P, :])
                             for i in range(ND)]
                prefetch.append(lambda: nc.sync.dma_start(
                    bgt[:], bvec["bg"].ap().rearrange("(j p) -> p j", p=P)))
                def tail(qq, K, hp, ctxs, es):
                    for si in range(2):
                        h = 2 * hp + si
                        ctx = ctxs[si]
                        if K == 1:
                            # re-inject the K=0 partial via a PE identity
                            # matmul so PV accumulates on top of it.
                            nc.tensor.matmul(
                                ctx[:], ident[0:HD + 1, 0:HD + 1],
                                ctx_sb[h][:], start=True, stop=False)
                        for kt4 in range(4):
                            nc.tensor.matmul(
                                ctx[:],
                                vsb[K * 4 + kt4][:, h * (HD + 1):(h + 1) * (HD + 1)],
                                es[si][:, kt4 * 256:(kt4 + 1) * 256],
                                start=(K == 0 and kt4 == 0), stop=(kt4 == 3))
                    for si in range(2):
                        h = 2 * hp + si
                        # evict to SBUF fast so the PSUM bank frees; the
                        # normalize chain runs entirely out of SBUF.
                        nc.vector.tensor_copy(ctx_sb[h][:], ctxs[si][:])
                        if K == 0:
                            continue
                        rowt = normp.tile([1, 256], F32, tag="rowt", name="rowt")
                        nc.vector.tensor_copy(
                            rowt[:], ctx_sb[h][HD:HD + 1, :])
                        rec = normp.tile([1, 256], F32, tag="rec", name="rec")
                        nc.vector.reciprocal_approx_fast(rec[:], rowt[:])
                        bc = normp.tile([HD, 256], F32, tag="bc", name="bc")
                        nc.gpsimd.partition_broadcast(bc[:], rec[:])
                        nc.vector.tensor_tensor(
                            ctxT[hp][si * HD:(si + 1) * HD, qq * 256:(qq + 1) * 256],
                            ctx_sb[h][0:HD, :], bc[:], OP.mult)

                pending = None
                for qq in range(4):
                    for K in range(2):
                        half = qq * 2 + K
                        if half == 0:
                            stg = stg0
                        else:
                            pool, tg = (stpA, "stA") if half % 2 == 0 else (stpB, "stB")
                            stg = []
                            for j in range(2):
                                st = pool.tile([P, KH], BF16, tag=tg, name=tg)
                                nc.sync.dma_start(
                                    st[:], biasb[(qq * 2 + j) * P:(qq * 2 + j + 1) * P,
                                                 K * KH:(K + 1) * KH])
                                stg.append(st.rearrange("p (k h) -> p k h", h=H))
                        # D/E weight prefetch rides BEHIND the bias tiles in
                        # the FIFO DMA queue so it can't delay them.
                        if half >= 2:
                            for _ in range(3 if half < 6 else 2):
                                if prefetch:
                                    prefetch.pop(0)()
                        for hp in range(6):
                            ss = [psS.tile([P, 1024], F32, tag="psS", name="psS")
                                  for _ in range(2)]
                            # head-PAIR qk matmuls interleaved on PE row
                            # groups 0/64: the two K=64 matmuls co-execute in
                            # the array AND keep the HAM activity monitor fed
                            # (single-row-group streams de-warm the clock).
                            for kt4 in range(4):
                                kt, off = K * 4 + kt4, kt4 * 256
                                for si in range(2):
                                    rp = si * HD
                                    # start=True only on the FIRST write to
                                    # each physical 2KB bank (kt4 even): it
                                    # clears has_written for the whole bank,
                                    # so the odd kt4 region (same bank) must
                                    # be a plain start=False fresh write or
                                    # the even region's bias would be lost.
                                    nc.tensor.matmul(
                                        ss[si][:, off:off + 256],
                                        kT[hp][rp:rp + HD, kt * P:(kt + 1) * P],
                                        qT[hp][rp:rp + HD, qq * 256:(qq + 1) * 256],
                                        start=(kt4 % 2 == 0), stop=False)
                            for si in range(2):
                                for kt4 in range(4):
                                    off = kt4 * 256
                                    for j in range(2):
                                        nc.tensor.matmul(
                                            ss[si][:, off + j * P: off + (j + 1) * P],
                                            stg[j][:, kt4 * P:(kt4 + 1) * P, 2 * hp + si],
                                            ident[:], start=False, stop=(j == 1))
                            es = []
                            for si in range(2):
                                e = expp.tile([P, 1024], BF16, tag="expT",
                                              name="expT", bufs=4)
                                nc.scalar.activation(e[:], ss[si][:], AF.Exp)
                                es.append(e)
                            ctxs = [psC.tile([HD + 1, 256], F32, tag="psC",
                                             name="psC") for _ in range(2)]
                            # software pipeline: emit the exp-dependent tail
                            # of the PREVIOUS pair here so the PE's in-order
                            # queue has fresh scores work while exp finishes.
                            if pending is not None:
                                tail(*pending)
                            pending = (qq, K, hp, ctxs, es)
                if pending is not None:
                    tail(*pending)

        # ---------------- Phase D/E: out-proj + GLU gate + combine ----------
        with tc.tile_pool(name="outb", bufs=3) as outb, \
             tc.tile_pool(name="psD", bufs=4, space="PSUM") as psD:
            # weights were prefetched during attention; interleave out-proj
            # and gate GEMMs per output chunk so stores start immediately.
            for jc in range(ND * 2):
                j, c = jc // 2, jc % 2
                sl = slice(c * 512, (c + 1) * 512)
                po = psD.tile([P, 512], F32, tag="psD", name="psD")
                for i in range(ND):
                    nc.tensor.matmul(po[:], wo[i][:, j * P:(j + 1) * P],
                                     ctxT[i][:, sl],
                                     start=(i == 0), stop=(i == ND - 1))
                ps = outb.tile([P, 512], F32, tag="posb", name="posb")
                nc.scalar.activation(ps[:], po[:], AF.Identity,
                                     bias=bvt["bo"][:, j:j + 1])
                pg = psD.tile([P, 512], F32, tag="psD", name="psD")
                for i in range(ND):
                    nc.tensor.matmul(pg[:], wg[i][:, j * P:(j + 1) * P],
                                     xsb[i][:, sl],
                                     start=(i == 0), stop=(i == ND - 1))
                th = outb.tile([P, 512], F32, tag="tanh", name="tanh")
                nc.scalar.activation(th[:], pg[:], AF.Tanh,
                                     bias=bgt[:, j:j + 1], scale=0.5)
                u = outb.tile([P, 512], F32, tag="u", name="u")
                nc.vector.scalar_tensor_tensor(
                    u[:], in0=th[:], scalar=1.0, in1=xsb[j][:, sl],
                    op0=OP.add, op1=OP.mult)
                fin = outb.tile([P, 512], F32, tag="fin", name="fin")
                nc.vector.scalar_tensor_tensor(
                    fin[:], in0=u[:], scalar=0.5, in1=ps[:],
                    op0=OP.mult, op1=OP.add)
                nc.sync.dma_start(outT[j * P:(j + 1) * P, sl], fin[:])


_cache = {}


def _build(debug=False):
    key = ("nc", debug)
    if key in _cache:
        return _cache[key]
    nc = bacc.Bacc("TRN2", target_bir_lowering=False, debug=False, num_devices=8)
    xT = nc.dram_tensor("xT", [D, N], BF16, kind="ExternalInput")
    biasb = nc.dram_tensor("biasb", [N, N * H], BF16, kind="ExternalInput")
    w = {nm: nc.dram_tensor(nm, [D, D], BF16, kind="ExternalInput")
         for nm in ("wq", "wk", "wv", "wg", "wo")}
    bvec = {nm: nc.dram_tensor(nm, [D], F32, kind="ExternalInput")
            for nm in ("bq", "bk", "bg", "bo")}
    outT = nc.dram_tensor("outT", [D, N], F32, kind="ExternalOutput")
    with tile.TileContext(nc) as tc:
        _emit(nc, tc, xT.ap(), biasb.ap(), {k: v.ap() for k, v in w.items()},
              bvec, outT.ap())
    nc.compile()
    _cache[key] = nc
    return nc


def _prep(inputs):
    scaling = HD ** (-0.5)
    shared = {
        "wq": _bf16(inputs["Wq"].T * scaling),
        "wk": _bf16(inputs["Wk"].T),
        "wv": _bf16(inputs["Wv"].T),
        "wg": _bf16(inputs["Wg"].T),
        "wo": _bf16(inputs["Wo"].T),
        "bq": np.ascontiguousarray(inputs["bq"] * scaling, np.float32),
        "bk": np.ascontiguousarray(inputs["bk"], np.float32),
        "bg": np.ascontiguousarray(inputs["bg"], np.float32),
        "bo": np.ascontiguousarray(
            inputs["bo"] + inputs["Wo"] @ inputs["bv"], np.float32),
    }
    ab = np.ascontiguousarray(inputs["attn_bias"], np.float32)
    nd = np.ascontiguousarray(inputs["ndata"], np.float32)
    in_maps = []
    for b in range(B):
        m = dict(shared)
        m["xT"] = _bf16(nd[b].T)
        m["biasb"] = ab[b].reshape(N, N * H).astype(ml_dtypes.bfloat16)
        in_maps.append(m)
    return in_maps


def run(inputs, trace=False, debug=False, **kw):
    nc = _build(debug=debug)
    in_maps = _prep(inputs)
    res = run_bass_kernel_spmd(nc, in_maps, core_ids=list(range(B)),
                               trace=trace, **kw)
    out = np.stack([np.ascontiguousarray(r["outT"].T) for r in res.results])
    return out, res


def kernel(**inputs):
    out, _ = run(inputs)
    return out



# revision 25
# speedup vs baseline: 1.1880x; 1.1880x over previous
"""BiasedMHA + GLU fused Trainium2 kernel.

Problem: out = GLU(x) + OutProj(MHA(x, attn_bias))  with
  B=8, N=1024, D=768, H=12, HD=64, fp32 inputs/outputs.

Strategy: data-parallel over batch across the 8 NeuronCores (one batch
element per core, no collectives). Per core everything is computed in a
"transposed" [channel, token] layout so every GEMM contracts the
partition dimension without any on-device activation transposes:

  xT [D, N] (host-pretransposed)    qT/kT = W.T-stationary GEMMs  [D, N]
  v via xT-stationary GEMM          -> natural [token, head*65] layout
  scoresT[k, q] = kT_h.T @ qT_h accumulated on top of PE-transposed
  attn_bias blocks (bf16 identity-matmuls straight into the scores PSUM)
  softmax over k (= partitions): no max-subtraction (|scores| <= ~8),
  denominator via an appended ones-column in v, applied after PV.
  Matmuls run in float32r (TF32-like, 11-bit mantissa, 4x faster than
  fp32 on the PE); attention processes HEAD PAIRS with the two K=64 qk
  matmuls interleaved on PE row groups 0/64 so they co-execute in the
  128x128 array and keep the HAM activity monitor fed (single-row-group
  streams let the PE clock-gate back to 1.2 GHz). Exp-dependent work is
  emitted one iteration late (software pipelining) so the in-order PE
  queue never head-of-line blocks on the ScalarE exp.

  Measured: HW exec ~286-288us per core (8 cores in parallel),
  relative error vs fp32 reference 6.2e-4.
"""

import os
import sys

for _p in ("/opt/trn_rl_repo", "/root/.axon_site/_ro/trn_rl_repo"):
    if os.path.isdir(_p) and _p not in sys.path:
        sys.path.insert(0, _p)

import numpy as np
import ml_dtypes

import concourse.bacc as bacc
import concourse.mybir as mybir
from concourse import tile
from concourse.bass_utils import run_bass_kernel_spmd
from concourse.masks import make_identity

B, N, D, H, HD = 8, 1024, 768, 12, 64
P = 128
ND = D // P           # 6 channel tiles
NN = N // P           # 8 token tiles
VW = H * (HD + 1)     # 780: v layout [token, h*(64+1)] with ones column

F32 = mybir.dt.float32
F32R = mybir.dt.float32r
BF16 = mybir.dt.bfloat16
AF = mybir.ActivationFunctionType
OP = mybir.AluOpType


def _bf16(x):
    return np.ascontiguousarray(x, dtype=np.float32).astype(ml_dtypes.bfloat16)


def _emit(nc, tc, xT, biasb, w, bvec, outT, dbg=None):
    KH = N * H // 2  # 6144 bf16 = half of k, all heads
    with tc.tile_pool(name="const", bufs=1) as constp, \
         tc.tile_pool(name="ctxT", bufs=1) as ctxp_sb, \
         tc.tile_pool(name="xp", bufs=1) as xp:

        ident = constp.tile([P, P], BF16, tag="ident", name="ident")
        make_identity(nc, ident[:])
        # PE warm-up: the first ~13us are DMA-latency-bound with zero PE
        # work, so the HAM clock-gate would hold the array at 1.2 GHz well
        # into phase B.  Burn idle cycles on an SBUF scratch tile so the
        # activity monitor un-throttles before the first real GEMM.
        wub = constp.tile([P, 512], BF16, tag="wub", name="wub")
        nc.vector.memset(wub[:], 0.0)
        with tc.tile_pool(name="psW", bufs=1, space="PSUM") as psW:
            wps = psW.tile([P, 512], F32, tag="psW", name="psW")
            for _ in range(10):
                nc.tensor.matmul(wps[:], ident[:], wub[:],
                                 start=True, stop=True, skip_group_check=True)
        bvt = {}
        for nm in ("bq", "bk", "bo"):
            t = constp.tile([P, ND], F32, tag=f"t{nm}", name=f"t{nm}")
            nc.sync.dma_start(t[:], bvec[nm].ap().rearrange("(j p) -> p j", p=P))
            bvt[nm] = t
        ones12 = constp.tile([P, H], F32, tag="ones12", name="ones12")
        nc.vector.memset(ones12[:], 1.0)

        ctxT = [ctxp_sb.tile([P, N], BF16, tag=f"cT{i}", name=f"cT{i}")
                for i in range(ND)]
        xsb = [xp.tile([P, N], BF16, tag=f"x{i}", name=f"x{i}")
               for i in range(ND)]
        # out-proj / gate weights: DMA'd here so they stream in during
        # attention; bf16 halves the traffic and SBUF footprint.
        wo = [ctxp_sb.tile([P, D], BF16, tag=f"wo{i}", name=f"wo{i}")
              for i in range(ND)]
        wg = [ctxp_sb.tile([P, D], BF16, tag=f"wg{i}", name=f"wg{i}")
              for i in range(ND)]
        bgt = constp.tile([P, ND], F32, tag="bg", name="bg")

        with tc.tile_pool(name="qkvT", bufs=1) as qkvp, \
             tc.tile_pool(name="ctxsb", bufs=1) as csbp, \
             tc.tile_pool(name="stA", bufs=3) as stpA:
            qT = [qkvp.tile([P, N], BF16, tag=f"qT{i}", name=f"qT{i}")
                  for i in range(ND)]
            kT = [qkvp.tile([P, N], BF16, tag=f"kT{i}", name=f"kT{i}")
                  for i in range(ND)]
            vsb = [qkvp.tile([P, VW], BF16, tag=f"v{t}", name=f"v{t}")
                   for t in range(NN)]
            vv = [t.rearrange("p (h c) -> p h c", c=HD + 1) for t in vsb]
            ctx_sb = [csbp.tile([HD + 1, 256], BF16, tag=f"cs{h}", name=f"cs{h}")
                      for h in range(12)]

            # ---------------- Phase B: q/k/v projections ----------------
            with tc.tile_pool(name="xTw", bufs=1) as xwp, \
                 tc.tile_pool(name="psB", bufs=2, space="PSUM") as psB:
                wsb = {nm: [xwp.tile([P, D], BF16, tag="wpool",
                                     name=f"{nm}{i}", bufs=12)
                            for i in range(ND)]
                       for nm in ("wq", "wk", "wv")}
                # critical-path DMAs first: the first q-GEMM group needs
                # wq[i]+x[i] pairs in order; bias staging has ~90us of slack.
                for i in range(ND):
                    nc.sync.dma_start(wsb["wq"][i][:], w["wq"][i * P:(i + 1) * P, :])
                    nc.sync.dma_start(xsb[i][:], xT[i * P:(i + 1) * P, :])
                for i in range(ND):
                    nc.sync.dma_start(wsb["wk"][i][:], w["wk"][i * P:(i + 1) * P, :])
                for i in range(ND):
                    nc.sync.dma_start(wsb["wv"][i][:], w["wv"][i * P:(i + 1) * P, :])
                stg0 = []
                for j in range(2):
                    st = stpA.tile([P, KH], BF16, tag="stA", name="stA")
                    nc.sync.dma_start(st[:], biasb[j * P:(j + 1) * P, 0:KH])
                    stg0.append(st.rearrange("p (k h) -> p k h", h=H))

                for nm, dst in (("wq", qT), ("wk", kT)):
                    for j in range(ND):
                        for c in range(2):
                            ps = psB.tile([P, 512], F32, tag="psB", name="psB")
                            for i in range(ND):
                                nc.tensor.matmul(
                                    ps[:], wsb[nm][i][:, j * P:(j + 1) * P],
                                    xsb[i][:, c * 512:(c + 1) * 512],
                                    start=(i == 0), stop=(i == ND - 1))
                            bt = bvt["bq" if nm == "wq" else "bk"]
                            nc.scalar.activation(dst[j][:, c * 512:(c + 1) * 512],
                                                 ps[:], AF.Identity,
                                                 bias=bt[:, j:j + 1])

                for t in range(NN):
                    nc.vector.tensor_copy(vv[t][:, :, HD], ones12[:])
                    for c, (lo, sz) in enumerate(((0, 512), (512, 256))):
                        ps = psB.tile([P, 512], F32, tag="psB", name="psB")
                        for i in range(ND):
                            nc.tensor.matmul(
                                ps[:, 0:sz], xsb[i][:, t * P:(t + 1) * P],
                                wsb["wv"][i][:, lo:lo + sz],
                                start=(i == 0), stop=(i == ND - 1))
                        h0 = lo // HD
                        nc.vector.tensor_copy(
                            vv[t][:, h0:h0 + sz // HD, 0:HD],
                            ps[:, 0:sz].rearrange("p (h c) -> p h c", c=HD))

            # ---------------- Phase C: attention ----------------
            # epochs = (qq quarter of q, K half of k); PV partials for K=0
            # park in ctx_sb, K=1 adds them back, normalizes, writes ctxT.
            with tc.tile_pool(name="stB", bufs=3) as stpB, \
                 tc.tile_pool(name="expT", bufs=3) as expp, \
                 tc.tile_pool(name="norm", bufs=2) as normp, \
                 tc.tile_pool(name="psS", bufs=3, space="PSUM") as psS, \
                 tc.tile_pool(name="psC", bufs=2, space="PSUM") as psC:
                # D/E weight prefetch, spread across attention epochs (the
                # sync DMA queue is FIFO: a burst here would delay the
                # just-in-time bias staging).
                prefetch = [lambda i=i: nc.sync.dma_start(
                                wo[i][:], w["wo"][i * P:(i + 1) * P, :])
                            for i in range(ND)]
                prefetch += [lambda i=i: nc.sync.dma_start(
                                 wg[i][:], w["wg"][i * P:(i + 1) * P, :])
                             for i in range(ND)]
                prefetch.append(lambda: nc.sync.dma_start(
                    bgt[:], bvec["bg"].ap().rearrange("(j p) -> p j", p=P)))
                def tail(qq, K, hp, ctxs, es):
                    for si in range(2):
                        h = 2 * hp + si
                        ctx = ctxs[si]
                        if K == 1:
                            # re-inject the K=0 partial via a PE identity
                            # matmul so PV accumulates on top of it.
                            nc.tensor.matmul(
                                ctx[:], ident[0:HD + 1, 0:HD + 1],
                                ctx_sb[h][:], start=True, stop=False)
                        for kt4 in range(4):
                            nc.tensor.matmul(
                                ctx[:],
                                vsb[K * 4 + kt4][:, h * (HD + 1):(h + 1) * (HD + 1)],
                                es[si][:, kt4 * 256:(kt4 + 1) * 256],
                                start=(K == 0 and kt4 == 0), stop=(kt4 == 3))
                    for si in range(2):
                        h = 2 * hp + si
                        # evict to SBUF fast so the PSUM bank frees; the
                        # normalize chain runs entirely out of SBUF.
                        nc.vector.tensor_copy(ctx_sb[h][:], ctxs[si][:])
                        if K == 0:
                            continue
                        rowt = normp.tile([1, 256], F32, tag="rowt", name="rowt")
                        nc.vector.tensor_copy(
                            rowt[:], ctx_sb[h][HD:HD + 1, :])
                        rec = normp.tile([1, 256], F32, tag="rec", name="rec")
                        nc.vector.reciprocal_approx_fast(rec[:], rowt[:])
                        bc = normp.tile([HD, 256], F32, tag="bc", name="bc")
                        nc.gpsimd.partition_broadcast(bc[:], rec[:])
                        nc.vector.tensor_tensor(
                            ctxT[hp][si * HD:(si + 1) * HD, qq * 256:(qq + 1) * 256],
                            ctx_sb[h][0:HD, :], bc[:], OP.mult)

                pending = None
                for qq in range(4):
                    for K in range(2):
                        half = qq * 2 + K
                        if half == 0:
                            stg = stg0
                        else:
                            pool, tg = (stpA, "stA") if half % 2 == 0 else (stpB, "stB")
                            stg = []
                            for j in range(2):
                                st = pool.tile([P, KH], BF16, tag=tg, name=tg)
                                nc.sync.dma_start(
                                    st[:], biasb[(qq * 2 + j) * P:(qq * 2 + j + 1) * P,
                                                 K * KH:(K + 1) * KH])
                                stg.append(st.rearrange("p (k h) -> p k h", h=H))
                        # D/E weight prefetch rides BEHIND the bias tiles in
                        # the FIFO DMA queue so it can't delay them.
                        if half >= 2:
                            for _ in range(3 if half < 6 else 2):
                                if prefetch:
                                    prefetch.pop(0)()
                        for hp in range(6):
                            ss = [psS.tile([P, 1024], F32, tag="psS", name="psS")
                                  for _ in range(2)]
                            # head-PAIR qk matmuls interleaved on PE row
                            # groups 0/64: the two K=64 matmuls co-execute in
                            # the array AND keep the HAM activity monitor fed
                            # (single-row-group streams de-warm the clock).
                            for kt4 in range(4):
                                kt, off = K * 4 + kt4, kt4 * 256
                                for si in range(2):
                                    rp = si * HD
                                    # start=True only on the FIRST write to
                                    # each physical 2KB bank (kt4 even): it
                                    # clears has_written for the whole bank,
                                    # so the odd kt4 region (same bank) must
                                    # be a plain start=False fresh write or
                                    # the even region's bias would be lost.
                                    nc.tensor.matmul(
                                        ss[si][:, off:off + 256],
                                        kT[hp][rp:rp + HD, kt * P:(kt + 1) * P],
                                        qT[hp][rp:rp + HD, qq * 256:(qq + 1) * 256],
                                        start=(kt4 % 2 == 0), stop=False)
                            for si in range(2):
                                for kt4 in range(4):
                                    off = kt4 * 256
                                    for j in range(2):
                                        nc.tensor.matmul(
                                            ss[si][:, off + j * P: off + (j + 1) * P],
                                            stg[j][:, kt4 * P:(kt4 + 1) * P, 2 * hp + si],
                                            ident[:], start=False, stop=(j == 1))
                            es = []
                            for si in range(2):
                                e = expp.tile([P, 1024], BF16, tag="expT",
                                              name="expT", bufs=4)
                                nc.scalar.activation(e[:], ss[si][:], AF.Exp)
                                es.append(e)
                            ctxs = [psC.tile([HD + 1, 256], F32, tag="psC",
                                             name="psC") for _ in range(2)]
                            # software pipeline: emit the exp-dependent tail
                            # of the PREVIOUS pair here so the PE's in-order
                            # queue has fresh scores work while exp finishes.
                            if pending is not None:
                                tail(*pending)
                            pending = (qq, K, hp, ctxs, es)
                if pending is not None:
                    tail(*pending)

        # ---------------- Phase D/E: out-proj + GLU gate + combine ----------
        with tc.tile_pool(name="outb", bufs=3) as outb, \
             tc.tile_pool(name="psD", bufs=4, space="PSUM") as psD:
            # weights were prefetched during attention; interleave out-proj
            # and gate GEMMs per output chunk so stores start immediately.
            for jc in range(ND * 2):
                j, c = jc // 2, jc % 2
                sl = slice(c * 512, (c + 1) * 512)
                po = psD.tile([P, 512], F32, tag="psD", name="psD")
                for i in range(ND):
                    nc.tensor.matmul(po[:], wo[i][:, j * P:(j + 1) * P],
                                     ctxT[i][:, sl],
                                     start=(i == 0), stop=(i == ND - 1))
                ps = outb.tile([P, 512], F32, tag="posb", name="posb")
                nc.scalar.activation(ps[:], po[:], AF.Identity,
                                     bias=bvt["bo"][:, j:j + 1])
                pg = psD.tile([P, 512], F32, tag="psD", name="psD")
                for i in range(ND):
                    nc.tensor.matmul(pg[:], wg[i][:, j * P:(j + 1) * P],
                                     xsb[i][:, sl],
                                     start=(i == 0), stop=(i == ND - 1))
                th = outb.tile([P, 512], F32, tag="tanh", name="tanh")
                nc.scalar.activation(th[:], pg[:], AF.Tanh,
                                     bias=bgt[:, j:j + 1], scale=0.5)
                u = outb.tile([P, 512], F32, tag="u", name="u")
                nc.vector.scalar_tensor_tensor(
                    u[:], in0=th[:], scalar=1.0, in1=xsb[j][:, sl],
                    op0=OP.add, op1=OP.mult)
                fin = outb.tile([P, 512], F32, tag="fin", name="fin")
                nc.vector.scalar_tensor_tensor(
                    fin[:], in0=u[:], scalar=0.5, in1=ps[:],
                    op0=OP.mult, op1=OP.add)
                nc.sync.dma_start(outT[j * P:(j + 1) * P, sl], fin[:])


_cache = {}


def _build(debug=False):
    key = ("nc", debug)
    if key in _cache:
        return _cache[key]
    nc = bacc.Bacc("TRN2", target_bir_lowering=False, debug=False, num_devices=8)
    xT = nc.dram_tensor("xT", [D, N], BF16, kind="ExternalInput")
    biasb = nc.dram_tensor("biasb", [N, N * H], BF16, kind="ExternalInput")
    w = {nm: nc.dram_tensor(nm, [D, D], BF16, kind="ExternalInput")
         for nm in ("wq", "wk", "wv", "wg", "wo")}
    bvec = {nm: nc.dram_tensor(nm, [D], F32, kind="ExternalInput")
            for nm in ("bq", "bk", "bg", "bo")}
    outT = nc.dram_tensor("outT", [D, N], F32, kind="ExternalOutput")
    with tile.TileContext(nc) as tc:
        _emit(nc, tc, xT.ap(), biasb.ap(), {k: v.ap() for k, v in w.items()},
              bvec, outT.ap())
    nc.compile()
    _cache[key] = nc
    return nc


def _prep(inputs):
    scaling = HD ** (-0.5)
    shared = {
        "wq": _bf16(inputs["Wq"].T * scaling),
        "wk": _bf16(inputs["Wk"].T),
        "wv": _bf16(inputs["Wv"].T),
        "wg": _bf16(inputs["Wg"].T),
        "wo": _bf16(inputs["Wo"].T),
        "bq": np.ascontiguousarray(inputs["bq"] * scaling, np.float32),
        "bk": np.ascontiguousarray(inputs["bk"], np.float32),
        "bg": np.ascontiguousarray(inputs["bg"], np.float32),
        "bo": np.ascontiguousarray(
            inputs["bo"] + inputs["Wo"] @ inputs["bv"], np.float32),
    }
    ab = np.ascontiguousarray(inputs["attn_bias"], np.float32)
    nd = np.ascontiguousarray(inputs["ndata"], np.float32)
    in_maps = []
    for b in range(B):
        m = dict(shared)
        m["xT"] = _bf16(nd[b].T)
        m["biasb"] = ab[b].reshape(N, N * H).astype(ml_dtypes.bfloat16)
        in_maps.append(m)
    return in_maps


def run(inputs, trace=False, debug=False, **kw):
    nc = _build(debug=debug)
    in_maps = _prep(inputs)
    res = run_bass_kernel_spmd(nc, in_maps, core_ids=list(range(B)),
                               trace=trace, **kw)
    out = np.stack([np.ascontiguousarray(r["outT"].T) for r in res.results])
    return out, res


def kernel(**inputs):
    out, _ = run(inputs)
    return out

